# revision 1
# baseline (speedup 1.0000x reference)
"""Trainium2 Bass kernel for nn_AttentionLayer (B=4, S=2048, H=12, D=64).

Sharding: 8 cores = 4 batches x 2 head-groups (6 heads each).
Per core: QKV projections for its 384 W-columns, then per-(head) attention
with a UniLM prefix "staircase" mask (cumsum of segment_ids is
non-decreasing, so each query attends to a prefix of keys). Fully-masked
[128k x 512q] tiles are skipped at program-build time (union over the 4
batches, so one SPMD program serves all cores); partially-masked tiles get
a multiplicative 0/1 mask after exp.

Layout choice: scores are computed transposed (k on partitions, q free) so
that probs can feed the ctx matmul as the moving operand with v as the
stationary operand. A ones-column appended to v accumulates the softmax
denominator for free; normalization is a per-q reciprocal broadcast. The
kernel returns ctx transposed ([384, 2048] per core); the host gather
transposes back.
"""

import sys

if "/opt/trn_rl_repo" not in sys.path:
    sys.path.insert(0, "/opt/trn_rl_repo")

from contextlib import ExitStack

import ml_dtypes
import numpy as np

import concourse.bass as bass
import concourse.mybir as mybir
import concourse.tile as tile
from concourse import bacc
from concourse.bass_utils import run_bass_kernel_spmd
B, S, W, H, D = 4, 2048, 768, 12, 64
NCORES = 8
HPC = 6  # heads per core
QB = 512  # q block (free dim of a scores tile)
KC = 128  # k chunk (partition dim of a scores tile)
NQB = S // QB
NKC = S // KC
MC = 3  # 128-row chunks of the 384 per-core W-columns
FKC = W // 128  # feature chunks (contraction for projections)
VW = HPC * (D + 1)  # v_aug row width per k-chunk (6 heads x (64 + ones col))
ACT_GROUP = 3  # k-chunks exp'd per ACT instruction (3 psum banks)

F32 = mybir.dt.float32
F32R = mybir.dt.float32r
BF16 = mybir.dt.bfloat16

TRACE = False  # set by test.py to profile
LAST_RESULTS = None  # BassKernelResults of the last run (for test.py)


def _ensure_ntff_hook():
    """This image's antenv lacks axon_hooks; register the ctypes NTFF
    profile hook from trn_agent_boot ourselves so trace=True works."""
    import types

    if "antenv.axon_hooks" in sys.modules:
        return
    try:
        from trn_agent_boot.trn_boot import _ntff_profile_via_ctypes

        hook = _ntff_profile_via_ctypes("/opt/axon/libaxon_pjrt.so")
    except Exception:
        hook = None
    mod = types.ModuleType("antenv.axon_hooks")
    mod._hook = hook
    mod.set_axon_ntff_profile_hook = lambda h: setattr(mod, "_hook", h)
    mod.get_axon_ntff_profile_hook = lambda: mod._hook
    sys.modules["antenv.axon_hooks"] = mod
    # artifact upload needs egress this sandbox doesn't have
    import concourse.bass_utils as _bu

    _bu.upload_artifacts = lambda d: "local://" + str(d)


def _classify(seg):
    """Union-over-batches tile classification from segment_ids.

    Returns (cumsums [B,S], per-qb visible k-chunk lists, boundary index).
    Element (k, q) is visible iff cs[k] <= cs[q]; cs is non-decreasing.
    """
    cs = np.cumsum(np.asarray(seg, np.int64), axis=1)
    vis_lists = [[] for _ in range(NQB)]
    bnd_index = {}
    for qb in range(NQB):
        for kc in range(NKC):
            any_computed = False
            all_full_vis = True
            for b in range(B):
                c = cs[b]
                full_mask = c[kc * KC] > c[qb * QB + QB - 1]
                full_vis = c[kc * KC + KC - 1] <= c[qb * QB]
                if not full_mask:
                    any_computed = True
                if not full_vis:
                    all_full_vis = False
            if any_computed:
                vis_lists[qb].append(kc)
                if not all_full_vis:
                    bnd_index[(kc, qb)] = len(bnd_index)
    return cs, vis_lists, bnd_index


def _build_program(vis_lists, bnd_index):
    nc = bacc.Bacc()
    n_bnd = max(len(bnd_index), 1)

    xT_d = nc.declare_dram_parameter("xT", [128, FKC * S], BF16, isOutput=False)
    wq_d = nc.declare_dram_parameter("wq", [128, FKC * HPC * D], BF16, isOutput=False)
    wk_d = nc.declare_dram_parameter("wk", [128, FKC * HPC * D], BF16, isOutput=False)
    wv_d = nc.declare_dram_parameter("wv", [128, FKC * HPC * D], BF16, isOutput=False)
    bqk_d = nc.declare_dram_parameter("bqk", [128, 2 * MC], F32, isOutput=False)
    bvb_d = nc.declare_dram_parameter("bvb", [128, HPC * D], F32, isOutput=False)
    csb_d = nc.declare_dram_parameter("cs_bcast", [128, S], F32, isOutput=False)
    csp_d = nc.declare_dram_parameter("cs_part", [128, NKC], F32, isOutput=False)
    out_d = nc.declare_dram_parameter("ctxT", [MC * 128, S], F32, isOutput=True)

    with ExitStack() as ctx:
        tc = ctx.enter_context(tile.TileContext(nc))
        persist = ctx.enter_context(tc.tile_pool(name="persist", bufs=1))

        qt = persist.tile([128, MC * S], BF16)
        kt = persist.tile([128, MC * S], BF16)
        v = persist.tile([128, NKC * VW], BF16)
        ctxt = persist.tile([128, MC * S], F32)
        msk = persist.tile([128, n_bnd * QB], BF16)
        cs_b = persist.tile([128, S], F32)
        cs_p = persist.tile([128, NKC], F32)
        bqk_sb = persist.tile([128, 2 * MC], F32)
        bv_sb = persist.tile([128, HPC * D], F32)
        ones_sb = persist.tile([1, 64], F32)
        nc.vector.memset(ones_sb, 1.0)
        nc.sync.dma_start(out=cs_b, in_=csb_d[:])
        nc.sync.dma_start(out=cs_p, in_=csp_d[:])
        nc.sync.dma_start(out=bqk_sb, in_=bqk_d[:])
        nc.sync.dma_start(out=bv_sb, in_=bvb_d[:])

        # 0/1 masks for boundary tiles, shared by all 6 heads of this core.
        for (kc, qb), bi in bnd_index.items():
            nc.vector.tensor_scalar(
                out=msk[:, bi * QB : (bi + 1) * QB],
                in0=cs_b[:, qb * QB : (qb + 1) * QB],
                scalar1=cs_p[:, kc : kc + 1],
                scalar2=None,
                op0=mybir.AluOpType.is_ge,
            )

        # ---- Phase A: projections ----
        with (
            tc.tile_pool(name="ld", bufs=1) as ld,
            tc.tile_pool(name="pps", bufs=3, space="PSUM") as pps,
        ):
            xt = ld.tile([128, FKC * S], BF16)
            wq_sb = ld.tile([128, FKC * HPC * D], BF16)
            wk_sb = ld.tile([128, FKC * HPC * D], BF16)
            wv_sb = ld.tile([128, FKC * HPC * D], BF16)
            nc.sync.dma_start(out=xt, in_=xT_d[:])
            nc.sync.dma_start(out=wq_sb, in_=wq_d[:])
            nc.sync.dma_start(out=wk_sb, in_=wk_d[:])
            nc.sync.dma_start(out=wv_sb, in_=wv_d[:])

            # ones columns of v_aug (overwritten nowhere below)
            v_ones = v.rearrange("p (s h e) -> p s h e", h=HPC, e=D + 1)[:, :, :, D : D + 1]
            nc.vector.memset(v_ones, 1.0)

            # qT, kT: [384, 2048] = W_slice^T @ x^T, stored bf16 head-pair-major
            for pi, (w_sb, out_sb) in enumerate(((wq_sb, qt), (wk_sb, kt))):
                for mc in range(MC):
                    for nb in range(NQB):
                        ps = pps.tile([128, QB], F32, tag="proj")
                        for kc in range(FKC):
                            nc.tensor.matmul(
                                ps,
                                lhsT=w_sb[:, kc * (HPC * D) + mc * 128 : kc * (HPC * D) + mc * 128 + 128],
                                rhs=xt[:, kc * S + nb * QB : kc * S + (nb + 1) * QB],
                                start=(kc == 0),
                                stop=(kc == FKC - 1),
                            )
                        # psum -> bf16 sbuf, adding the (per-partition) bias
                        # on DVE (keeps ACT free for the exp stream later)
                        nc.vector.tensor_scalar_add(
                            out_sb[:, mc * S + nb * QB : mc * S + (nb + 1) * QB],
                            ps,
                            bqk_sb[:, pi * MC + mc : pi * MC + mc + 1],
                        )

            # v: [2048, 384] = x @ Wv_slice, natural layout, interleaved ones col
            for sc in range(NKC):
                ps = pps.tile([128, HPC * D], F32, tag="proj")
                for kc in range(FKC):
                    nc.tensor.matmul(
                        ps,
                        lhsT=xt[:, kc * S + sc * KC : kc * S + sc * KC + KC],
                        rhs=wv_sb[:, kc * (HPC * D) : (kc + 1) * (HPC * D)],
                        start=(kc == 0),
                        stop=(kc == FKC - 1),
                    )
                dest = v.rearrange("p (s h e) -> p s h e", h=HPC, e=D + 1)[
                    :, sc, :, 0:D
                ]
                nc.vector.tensor_add(
                    dest,
                    ps.rearrange("p (h e) -> p h e", e=D),
                    bv_sb.rearrange("p (h e) -> p h e", e=D),
                )

        # ---- Phase B: attention ----
        # Heads are processed in (even, odd) pairs: their qT/kT rows live on
        # partitions 0-63 / 64-127, so interleaved scores matmuls land on
        # disjoint PE row groups and run concurrently (auto tile_position
        # from base_partition). The softmax drain copies ctx PSUM to SBUF
        # immediately (freeing the bank), computes 1/l with the fast DVE
        # reciprocal, and broadcasts it across 64 partitions via a DRAM
        # round-trip DMA (SBUF APs cannot have partition stride 0).
        with (
            tc.tile_pool(name="scps", bufs=2, space="PSUM") as scps,
            tc.tile_pool(name="ctxps", bufs=2, space="PSUM") as ctxps,
            tc.tile_pool(name="expp", bufs=3) as expp,
            tc.tile_pool(name="drainp", bufs=4) as small,
        ):
            for hp in range(HPC // 2):
                mcq = hp
                for qb in range(NQB):
                    vis = vis_lists[qb]
                    groups = [vis[i : i + ACT_GROUP] for i in range(0, len(vis), ACT_GROUP)]
                    n_mm = len(vis)
                    cps = {}
                    for par in range(2):
                        cps[par] = ctxps.tile([65, QB], F32, tag="cps", name=f"cps{par}")
                    mm_i = 0
                    for g in groups:
                        sps = {}
                        esb = {}
                        for par in range(2):
                            sps[par] = scps.tile([128, ACT_GROUP * QB], F32, tag="sps", name=f"sps{par}")
                            esb[par] = expp.tile([128, ACT_GROUP * QB], BF16, tag="esb", name=f"esb{par}")
                        for j, kc in enumerate(g):
                            for par in range(2):
                                po = par * 64
                                nc.tensor.matmul(
                                    sps[par][:, j * QB : (j + 1) * QB],
                                    lhsT=kt[po : po + 64, mcq * S + kc * KC : mcq * S + kc * KC + KC],
                                    rhs=qt[po : po + 64, mcq * S + qb * QB : mcq * S + (qb + 1) * QB],
                                    start=True,
                                    stop=True,
                                )
                        n = len(g) * QB
                        for par in range(2):
                            nc.scalar.activation(
                                out=esb[par][:, :n],
                                in_=sps[par][:, :n],
                                func=mybir.ActivationFunctionType.Exp,
                                scale=1.0 / float(np.sqrt(np.float32(D))),
                            )
                        for j, kc in enumerate(g):
                            bi = bnd_index.get((kc, qb))
                            for par in range(2):
                                h = 2 * hp + par
                                if bi is not None:
                                    nc.vector.tensor_mul(
                                        esb[par][:, j * QB : (j + 1) * QB],
                                        esb[par][:, j * QB : (j + 1) * QB],
                                        msk[:, bi * QB : (bi + 1) * QB],
                                    )
                                nc.tensor.matmul(
                                    cps[par],
                                    lhsT=v[:, kc * VW + h * (D + 1) : kc * VW + (h + 1) * (D + 1)],
                                    rhs=esb[par][:, j * QB : (j + 1) * QB],
                                    start=(mm_i == 0),
                                    stop=(mm_i == n_mm - 1),
                                )
                            mm_i += 1
                    # drain: normalize ctx[d, q] by 1/l[q] (l in row 64)
                    for par in range(2):
                        h = 2 * hp + par
                        po = par * 64
                        cbuf = small.tile([65, QB], F32, tag="cbuf")
                        nc.vector.tensor_copy(cbuf, cps[par])
                        rc = small.tile([1, QB], F32, tag="rc")
                        nc.vector.reciprocal(rc, cbuf[64:65, :])
                        rb = ctxps.tile([64, QB], F32, tag="cps", name=f"rb{par}")
                        nc.tensor.matmul(rb, lhsT=ones_sb, rhs=rc, start=True, stop=True)
                        rb_sb = small.tile([64, QB], F32, tag="rb", name=f"rbs{par}")
                        nc.vector.tensor_copy(rb_sb, rb)
                        nc.vector.tensor_mul(
                            ctxt[po : po + 64, mcq * S + qb * QB : mcq * S + (qb + 1) * QB],
                            cbuf[0:64, :],
                            rb_sb,
                        )

        for t in range(MC):
            nc.sync.dma_start(
                out=out_d[t * 128 : (t + 1) * 128, :], in_=ctxt[:, t * S : (t + 1) * S]
            )

    nc.finalize()
    return nc


def _core_inputs(x, segment_ids, Wq, bq, Wk, bk, Wv, bv, cs, core):
    b, h0 = core // 2, HPC * (core % 2)
    cols = slice(h0 * D, (h0 + HPC) * D)
    xT = np.ascontiguousarray(x[b].T)  # [768, 2048]
    xT_s = (
        xT.reshape(FKC, 128, S).transpose(1, 0, 2).reshape(128, FKC * S)
    ).astype(ml_dtypes.bfloat16)

    def wprep(Wm):
        ws = Wm[:, cols]  # [768, 384]
        return np.ascontiguousarray(
            ws.reshape(FKC, 128, HPC * D).transpose(1, 0, 2).reshape(128, FKC * HPC * D)
        ).astype(ml_dtypes.bfloat16)

    bq_s = np.ascontiguousarray(bq[cols].reshape(MC, 128).T)
    bk_s = np.ascontiguousarray(bk[cols].reshape(MC, 128).T)
    bqk = np.concatenate([bq_s, bk_s], axis=1)  # [128, 6]
    bvb = np.ascontiguousarray(np.broadcast_to(bv[cols], (128, HPC * D)))
    csf = cs[b].astype(np.float32)
    cs_bcast = np.ascontiguousarray(np.broadcast_to(csf, (128, S)))
    cs_part = np.ascontiguousarray(csf.reshape(NKC, KC).T)
    return {
        "xT": np.ascontiguousarray(xT_s),
        "wq": wprep(Wq),
        "wk": wprep(Wk),
        "wv": wprep(Wv),
        "bqk": np.ascontiguousarray(bqk),
        "bvb": bvb,
        "cs_bcast": cs_bcast,
        "cs_part": cs_part,
    }


def kernel(x, segment_ids, Wq, bq, Wk, bk, Wv, bv):
    global LAST_RESULTS
    x = np.asarray(x, np.float32)
    segment_ids = np.asarray(segment_ids)
    Wq, bq = np.asarray(Wq, np.float32), np.asarray(bq, np.float32)
    Wk, bk = np.asarray(Wk, np.float32), np.asarray(bk, np.float32)
    Wv, bv = np.asarray(Wv, np.float32), np.asarray(bv, np.float32)

    cs, vis_lists, bnd_index = _classify(segment_ids)
    nc = _build_program(vis_lists, bnd_index)
    in_maps = [
        _core_inputs(x, segment_ids, Wq, bq, Wk, bk, Wv, bv, cs, c)
        for c in range(NCORES)
    ]
    if TRACE:
        _ensure_ntff_hook()
    res = run_bass_kernel_spmd(nc, in_maps, list(range(NCORES)), trace=TRACE)
    LAST_RESULTS = res

    out = np.empty((B, S, W), np.float32)
    for c in range(NCORES):
        b, h0 = c // 2, HPC * (c % 2)
        out[b, :, h0 * D : (h0 + HPC) * D] = res.results[c]["ctxT"].T
    return out



# revision 2
# speedup vs baseline: 1.2883x; 1.2883x over previous
"""Trainium2 Bass kernel for nn_AttentionLayer (B=4, S=2048, H=12, D=64).

Sharding: 8 cores = 4 batches x 2 head-groups (6 heads each).

v2 design (vs the v1 baseline at ~371us):
- All QKV projections run upfront as a dense PE phase (fat 3-bank PSUM
  pool), with S-major chunked input DMAs so the first matmul starts ~2us in.
- Attention per head-pair: scores stay [k_part, q_free] with the two heads
  of a pair on PE row halves (concurrent matmuls), but the ctx matmul is
  FLIPPED: exp'd probs are the stationary operand (128-col chunks, FWL) and
  v_aug=[v|ones] is the moving operand, so ctx lands [q_part, d_free] and
  the softmax denominator (ones column) is per-PARTITION. The drain then
  needs only a [128,4] reciprocal_approx_fast + 4 per-partition scalar
  muls instead of v1's 3.3us single-lane reciprocal + broadcast matmul.
- ctx accumulates chunk-interleaved into ONE PSUM bank per head with a
  single start=True (first write to each element overwrites: has_written
  semantics), so PSUM fits exactly: 2 tags x 3 banks scores + 2 x 1 ctx.
- Boundary-tile 0/1 masks are built on the host, DMA'd, and applied
  post-exp on the (otherwise idle) GpSimd engine.
- Output [q,d]-oriented, DMA'd out per q-block; host reassembles.
"""

import sys

if "/opt/trn_rl_repo" not in sys.path:
    sys.path.insert(0, "/opt/trn_rl_repo")

from contextlib import ExitStack

import ml_dtypes
import numpy as np

import concourse.bass as bass
import concourse.mybir as mybir
import concourse.tile as tile
from concourse import bacc
from concourse.bass_utils import run_bass_kernel_spmd

B, S, W, H, D = 4, 2048, 768, 12, 64
NCORES = 8
HPC = 6  # heads per core
QB = 512  # q block (free dim of a scores tile)
KC = 128  # k chunk (partition dim of a scores tile)
NQB = S // QB
NKC = S // KC
NB = 4  # S-chunks for x DMA / projection tiling (512 each)
MC = 3  # 128-row chunks of the 384 per-core W-columns (head pairs)
FKC = W // 128  # feature chunks (contraction for projections)
EW = D + 1  # per-head v_aug width (64 v cols + ones col)
VW = HPC * EW  # v_aug row width per k-chunk
G = 3  # k-chunks exp'd per ACT instruction (3 psum banks)

F32 = mybir.dt.float32
BF16 = mybir.dt.bfloat16

TRACE = False  # set by test.py to profile
LAST_RESULTS = None  # BassKernelResults of the last run (for test.py)


def _ensure_ntff_hook():
    """This image's antenv lacks axon_hooks; register the ctypes NTFF
    profile hook from trn_agent_boot ourselves so trace=True works."""
    import types

    if "antenv.axon_hooks" in sys.modules:
        return
    try:
        from trn_agent_boot.trn_boot import _ntff_profile_via_ctypes

        hook = _ntff_profile_via_ctypes("/opt/axon/libaxon_pjrt.so")
    except Exception:
        hook = None
    mod = types.ModuleType("antenv.axon_hooks")
    mod._hook = hook
    mod.set_axon_ntff_profile_hook = lambda h: setattr(mod, "_hook", h)
    mod.get_axon_ntff_profile_hook = lambda: mod._hook
    sys.modules["antenv.axon_hooks"] = mod
    # artifact upload needs egress this sandbox doesn't have
    import concourse.bass_utils as _bu

    _bu.upload_artifacts = lambda d: "local://" + str(d)


def _classify(seg):
    """Union-over-batches tile classification from segment_ids.

    Returns (cumsums [B,S], per-qb visible k-chunk lists, boundary index).
    Element (k, q) is visible iff cs[k] <= cs[q]; cs is non-decreasing.
    """
    cs = np.cumsum(np.asarray(seg, np.int64), axis=1)
    vis_lists = [[] for _ in range(NQB)]
    bnd_index = {}
    for qb in range(NQB):
        for kc in range(NKC):
            any_computed = False
            all_full_vis = True
            for b in range(B):
                c = cs[b]
                full_mask = c[kc * KC] > c[qb * QB + QB - 1]
                full_vis = c[kc * KC + KC - 1] <= c[qb * QB]
                if not full_mask:
                    any_computed = True
                if not full_vis:
                    all_full_vis = False
            if any_computed:
                vis_lists[qb].append(kc)
                if not all_full_vis:
                    bnd_index[(kc, qb)] = len(bnd_index)
    return cs, vis_lists, bnd_index


def _build_program(vis_lists, bnd_index):
    nc = bacc.Bacc()
    n_bnd = max(len(bnd_index), 1)

    xt_d = nc.declare_dram_parameter("xT", [128, NB * FKC * QB], BF16, isOutput=False)
    wq_d = nc.declare_dram_parameter("wq", [128, MC * FKC * 128], BF16, isOutput=False)
    wk_d = nc.declare_dram_parameter("wk", [128, MC * FKC * 128], BF16, isOutput=False)
    wv_d = nc.declare_dram_parameter("wv", [128, FKC * HPC * D], BF16, isOutput=False)
    bqk_d = nc.declare_dram_parameter("bqk", [128, 2 * MC], F32, isOutput=False)
    bvb_d = nc.declare_dram_parameter("bvb", [128, HPC * D], F32, isOutput=False)
    msk_d = nc.declare_dram_parameter("msk", [128, n_bnd * QB], BF16, isOutput=False)
    # output: ctx in [q_part, (qchunk, head, d)] layout, 16 chunks of 128 q
    out_d = nc.declare_dram_parameter("ctx", [128, 4 * NQB * HPC * D], F32, isOutput=True)

    with ExitStack() as ctx:
        tc = ctx.enter_context(tile.TileContext(nc))
        persist = ctx.enter_context(tc.tile_pool(name="persist", bufs=1))

        qt = persist.tile([128, MC * S], BF16)
        kt = persist.tile([128, MC * S], BF16)
        v = persist.tile([128, NKC * VW], BF16)
        ctxq = persist.tile([128, 4 * NQB * HPC * D], F32)
        msk = persist.tile([128, n_bnd * QB], BF16)
        bqk_sb = persist.tile([128, 2 * MC], F32)
        bv_sb = persist.tile([128, HPC * D], F32)
        xt = persist.tile([128, NB * FKC * QB], BF16)
        wq_sb = persist.tile([128, MC * FKC * 128], BF16)
        wk_sb = persist.tile([128, MC * FKC * 128], BF16)
        wv_sb = persist.tile([128, FKC * HPC * D], BF16)

        # DMAs, ordered so the first projection tiles unblock earliest.
        XB = FKC * QB  # xt columns per S-chunk
        WB = FKC * 128  # wq/wk columns per mc chunk
        nc.sync.dma_start(out=xt[:, 0:XB], in_=xt_d[:, 0:XB])
        nc.sync.dma_start(out=wq_sb[:, 0:WB], in_=wq_d[:, 0:WB])
        nc.sync.dma_start(out=wk_sb[:, 0:WB], in_=wk_d[:, 0:WB])
        nc.sync.dma_start(out=wv_sb, in_=wv_d[:])
        nc.sync.dma_start(out=bqk_sb, in_=bqk_d[:])
        nc.sync.dma_start(out=bv_sb, in_=bvb_d[:])
        for nb in range(1, NB):
            nc.sync.dma_start(
                out=xt[:, nb * XB : (nb + 1) * XB], in_=xt_d[:, nb * XB : (nb + 1) * XB]
            )
        nc.sync.dma_start(out=msk, in_=msk_d[:])
        for mc in range(1, MC):
            nc.sync.dma_start(
                out=wq_sb[:, mc * WB : (mc + 1) * WB], in_=wq_d[:, mc * WB : (mc + 1) * WB]
            )
            nc.sync.dma_start(
                out=wk_sb[:, mc * WB : (mc + 1) * WB], in_=wk_d[:, mc * WB : (mc + 1) * WB]
            )

        # ones columns of v_aug (overwritten nowhere below)
        v_ones = v.rearrange("p (s h e) -> p s h e", h=HPC, e=EW)[:, :, :, D : D + 1]
        nc.vector.memset(v_ones, 1.0)

        def xt_ap(nb, fkc, c0, c1):
            base = nb * XB + fkc * QB
            return xt[:, base + c0 : base + c1]

        # ---- Phase A: all projections upfront (PE-dense) ----
        with tc.tile_pool(name="fatproj", bufs=3, space="PSUM") as fat:

            def proj_qk(w_sb, out_sb, pi, mc, nb):
                ps = fat.tile([128, QB], F32, tag="fp")
                for fkc in range(FKC):
                    nc.tensor.matmul(
                        ps,
                        lhsT=w_sb[:, mc * WB + fkc * 128 : mc * WB + (fkc + 1) * 128],
                        rhs=xt_ap(nb, fkc, 0, QB),
                        start=(fkc == 0),
                        stop=(fkc == FKC - 1),
                    )
                nc.vector.tensor_scalar_add(
                    out_sb[:, mc * S + nb * QB : mc * S + (nb + 1) * QB],
                    ps,
                    bqk_sb[:, pi * MC + mc : pi * MC + mc + 1],
                )

            def proj_v(sc):
                nb, i = sc // 4, sc % 4
                ps = fat.tile([128, HPC * D], F32, tag="fp")
                for fkc in range(FKC):
                    nc.tensor.matmul(
                        ps,
                        lhsT=xt_ap(nb, fkc, i * 128, (i + 1) * 128),
                        rhs=wv_sb[:, fkc * (HPC * D) : (fkc + 1) * (HPC * D)],
                        start=(fkc == 0),
                        stop=(fkc == FKC - 1),
                    )
                dest = v.rearrange("p (s h e) -> p s h e", h=HPC, e=EW)[:, sc, :, 0:D]
                nc.vector.tensor_add(
                    dest,
                    ps.rearrange("p (h e) -> p h e", e=D),
                    bv_sb.rearrange("p (h e) -> p h e", e=D),
                )

            # mc0 q/k + v, in nb order (chunked DMAs unblock progressively)
            for nb in range(NB):
                proj_qk(wq_sb, qt, 0, 0, nb)
                proj_qk(wk_sb, kt, 1, 0, nb)
                for i in range(4):
                    proj_v(nb * 4 + i)
            for mc in range(1, MC):
                for nb in range(NB):
                    proj_qk(wq_sb, qt, 0, mc, nb)
                    proj_qk(wk_sb, kt, 1, mc, nb)

        # ---- Phase B: attention ----
        # Per head-pair hp (heads 2hp, 2hp+1 on qt/kt partitions 0-63 /
        # 64-127): scores land [k_part, q_free] in psum, exp'd by ACT in
        # groups of G k-chunks to bf16 SBUF, boundary tiles masked on
        # GpSimd, then the FLIPPED ctx matmul (esb chunks stationary, v_aug
        # moving) accumulates ctx [q_part, 65] per q-chunk into one psum
        # bank per head (single start=True; first write to each element
        # overwrites since its has_written bit is clear).
        with (
            tc.tile_pool(name="scps", bufs=1, space="PSUM") as scps,
            tc.tile_pool(name="ctxps", bufs=1, space="PSUM") as ctxps,
            tc.tile_pool(name="expp", bufs=3) as expp,
            tc.tile_pool(name="drainp", bufs=2) as drainp,
        ):
            for hp in range(MC):
                for qb in range(NQB):
                    vis = vis_lists[qb]
                    groups = [vis[i : i + G] for i in range(0, len(vis), G)]
                    cps = {
                        par: ctxps.tile([128, 4 * EW], F32, tag=f"c{par}", name=f"cps{par}")
                        for par in range(2)
                    }
                    first = True
                    n_done = 0
                    for g in groups:
                        sps = {}
                        esb = {}
                        for par in range(2):
                            sps[par] = scps.tile(
                                [128, G * QB], F32, tag=f"s{par}", name=f"sps{par}"
                            )
                            esb[par] = expp.tile(
                                [128, G * QB], BF16, tag=f"e{par}", name=f"esb{par}"
                            )
                        for j, kc in enumerate(g):
                            for par in range(2):
                                po = par * 64
                                nc.tensor.matmul(
                                    sps[par][:, j * QB : (j + 1) * QB],
                                    lhsT=kt[po : po + 64, hp * S + kc * KC : hp * S + kc * KC + KC],
                                    rhs=qt[po : po + 64, hp * S + qb * QB : hp * S + (qb + 1) * QB],
                                    start=True,
                                    stop=True,
                                )
                        n = len(g) * QB
                        for par in range(2):
                            nc.scalar.activation(
                                out=esb[par][:, :n],
                                in_=sps[par][:, :n],
                                func=mybir.ActivationFunctionType.Exp,
                                scale=0.125,
                            )
                        # boundary masks (0/1, post-exp) on GpSimd
                        for j, kc in enumerate(g):
                            bi = bnd_index.get((kc, qb))
                            if bi is not None:
                                for par in range(2):
                                    nc.gpsimd.tensor_mul(
                                        esb[par][:, j * QB : (j + 1) * QB],
                                        esb[par][:, j * QB : (j + 1) * QB],
                                        msk[:, bi * QB : (bi + 1) * QB],
                                    )
                        # flipped ctx: esb chunk stationary, v_aug moving
                        n_done += len(g)
                        last_g = n_done == len(vis)
                        for j, kc in enumerate(g):
                            last_kc = last_g and (j == len(g) - 1)
                            for par in range(2):
                                hg = 2 * hp + par
                                for ch in range(4):
                                    nc.tensor.matmul(
                                        cps[par][:, ch * EW : (ch + 1) * EW],
                                        lhsT=esb[par][:, j * QB + ch * 128 : j * QB + (ch + 1) * 128],
                                        rhs=v[:, kc * VW + hg * EW : kc * VW + (hg + 1) * EW],
                                        start=(first and ch == 0),
                                        stop=(last_kc and ch == 3),
                                        skip_group_check=True,
                                    )
                                if par == 1:
                                    first = False
                    # drain: per head, copy bank out, recip the 4 denom
                    # cols, scale each q-chunk by its per-partition 1/l
                    for par in range(2):
                        hg = 2 * hp + par
                        cb = drainp.tile([128, 4 * EW], F32, tag=f"cb{par}", name=f"cb{par}")
                        nc.vector.tensor_copy(cb, cps[par])
                        rc = drainp.tile([128, 4], F32, tag=f"rc{par}", name=f"rc{par}")
                        cb_v = cb.rearrange("p (c e) -> p c e", e=EW)
                        nc.vector.tensor_copy(rc, cb_v[:, :, D : D + 1])
                        ri = drainp.tile([128, 4], F32, tag=f"ri{par}", name=f"ri{par}")
                        nc.vector.reciprocal_approx_fast(out=ri, in_=rc)
                        for ch in range(4):
                            col = (qb * 4 + ch) * (HPC * D) + hg * D
                            nc.vector.tensor_scalar_mul(
                                ctxq[:, col : col + D],
                                cb[:, ch * EW : ch * EW + D],
                                ri[:, ch : ch + 1],
                            )
                    if hp == MC - 1:
                        cpq = 4 * HPC * D  # output columns per qb
                        nc.sync.dma_start(
                            out=out_d[:, qb * cpq : (qb + 1) * cpq],
                            in_=ctxq[:, qb * cpq : (qb + 1) * cpq],
                        )

    nc.finalize()
    return nc


def _core_inputs(x, segment_ids, Wq, bq, Wk, bk, Wv, bv, cs, bnd_index, core):
    b, h0 = core // 2, HPC * (core % 2)
    cols = slice(h0 * D, (h0 + HPC) * D)
    xT = np.ascontiguousarray(x[b].T)  # [768, 2048]
    # [128, nb, fkc, 512] layout: S-major chunks, feature-chunk minor
    xt_s = (
        xT.reshape(FKC, 128, NB, QB).transpose(1, 2, 0, 3).reshape(128, NB * FKC * QB)
    ).astype(ml_dtypes.bfloat16)

    def wqk_prep(Wm):
        ws = Wm[:, cols]  # [768, 384]
        return np.ascontiguousarray(
            ws.reshape(FKC, 128, MC, 128).transpose(1, 2, 0, 3).reshape(128, MC * FKC * 128)
        ).astype(ml_dtypes.bfloat16)

    ws = Wv[:, cols]
    wv_s = np.ascontiguousarray(
        ws.reshape(FKC, 128, HPC * D).transpose(1, 0, 2).reshape(128, FKC * HPC * D)
    ).astype(ml_dtypes.bfloat16)

    bq_s = np.ascontiguousarray(bq[cols].reshape(MC, 128).T)
    bk_s = np.ascontiguousarray(bk[cols].reshape(MC, 128).T)
    bqk = np.concatenate([bq_s, bk_s], axis=1)  # [128, 6]
    bvb = np.ascontiguousarray(np.broadcast_to(bv[cols], (128, HPC * D)))

    csb = cs[b]
    n_bnd = max(len(bnd_index), 1)
    mskv = np.zeros((128, n_bnd * QB), np.float32)
    for (kc, qb), bi in bnd_index.items():
        mskv[:, bi * QB : (bi + 1) * QB] = (
            csb[kc * KC : (kc + 1) * KC, None] <= csb[None, qb * QB : (qb + 1) * QB]
        )
    return {
        "xT": np.ascontiguousarray(xt_s),
        "wq": wqk_prep(Wq),
        "wk": wqk_prep(Wk),
        "wv": wv_s,
        "bqk": np.ascontiguousarray(bqk),
        "bvb": bvb,
        "msk": mskv.astype(ml_dtypes.bfloat16),
    }


def kernel(x, segment_ids, Wq, bq, Wk, bk, Wv, bv):
    global LAST_RESULTS
    x = np.asarray(x, np.float32)
    segment_ids = np.asarray(segment_ids)
    Wq, bq = np.asarray(Wq, np.float32), np.asarray(bq, np.float32)
    Wk, bk = np.asarray(Wk, np.float32), np.asarray(bk, np.float32)
    Wv, bv = np.asarray(Wv, np.float32), np.asarray(bv, np.float32)

    cs, vis_lists, bnd_index = _classify(segment_ids)
    nc = _build_program(vis_lists, bnd_index)
    in_maps = [
        _core_inputs(x, segment_ids, Wq, bq, Wk, bk, Wv, bv, cs, bnd_index, c)
        for c in range(NCORES)
    ]
    if TRACE:
        _ensure_ntff_hook()
    res = run_bass_kernel_spmd(nc, in_maps, list(range(NCORES)), trace=TRACE)
    LAST_RESULTS = res

    out = np.empty((B, S, W), np.float32)
    for c in range(NCORES):
        b, h0 = c // 2, HPC * (c % 2)
        # [128, 16, 384] -> [16, 128, 384] -> [2048, 384]
        cq = res.results[c]["ctx"].reshape(128, 4 * NQB, HPC * D)
        out[b, :, h0 * D : (h0 + HPC) * D] = cq.transpose(1, 0, 2).reshape(S, HPC * D)
    return out
